# revision 1
# baseline (speedup 1.0000x reference)
"""Trainium2 Bass kernel for pairwise-MLP GNN message passing.

Computation (per batch b, position l):
    x[i,j] = concat(states[l,i], states[l,j])           # [N,N,2D]
    out    = sigmoid(MLP(x))                            # [N,N,8], MLP: 32->64->64->8

Factorization used on device: the first linear layer splits into
A = states @ W1[:D] + b1 and B = states @ W1[D:], so
h1[i,j] = relu(A[i] + B[j]) — the N^2 expansion happens as a cheap
broadcast add on the vector engine instead of an N^2-row matmul.

Sharding: data-parallel over batch, core c <- batch c (8 cores, B=8).

Device layout (per core, L=64 l-blocks, 2 l-blocks = 1 "sb" superblock,
2 sbs = 1 "pair"):
  - features live on partitions: partitions 0:64 = even l-block of the sb,
    64:128 = odd l-block (via a host-side shifted copy of states^T feeding
    block-diagonal-packed matmuls).
  - pair columns col = 32*i + j, 1024 per l-block.
  - L2/L3 run as concurrent 64x64 / 64x32 tile_position matmuls.
  - Output leaves the device as sigmoid(z) in [16 pairs, 4 groups, 8 f, 1024]
    per core; the host inverts the layout while unsharding.
"""

import os
import sys

import numpy as np

for _p in ("/opt/trn_rl_repo", "/root/.axon_site/_ro/trn_rl_repo"):
    if os.path.isdir(_p) and _p not in sys.path:
        sys.path.insert(0, _p)

from concourse import bacc, mybir, tile
from concourse.bass_utils import run_bass_kernel_spmd

B, L, N, D = 8, 64, 32, 16
H = 64            # hidden width (h1 and h2)
F = 8             # out_dim
NCORES = 8
NSB = L // 2      # 32 superblocks per core
NPAIR = NSB // 2  # 16 pairs per core
COLS = N * N      # 1024 pair columns per l-block
S2_ACT_SPLIT = 576  # columns of the h2 eviction done on ScalarE (rest on VectorE)

FP32 = mybir.dt.float32
BF16 = mybir.dt.bfloat16
NP_BF16 = mybir.dt.np(BF16)

_PROGRAM = None  # (nc, input_names)
LAST_RESULT = None  # BassKernelResults of the most recent kernel() call


def _build_program():
    nc = bacc.Bacc("TRN2", target_bir_lowering=False, debug=False)

    d_statesQ = nc.dram_tensor("statesQ", [64, 2048], BF16, kind="ExternalInput").ap()
    d_Wl1 = nc.dram_tensor("Wl1", [48, 128], BF16, kind="ExternalInput").ap()
    d_W23 = nc.dram_tensor("W23", [128, 96], BF16, kind="ExternalInput").ap()
    d_biases = nc.dram_tensor("biases", [128, 3], FP32, kind="ExternalInput").ap()
    d_out = nc.dram_tensor(
        "out", [NPAIR // 2, 4, F, 2, COLS], FP32, kind="ExternalOutput"
    ).ap()

    add = mybir.AluOpType.add
    max_ = mybir.AluOpType.max
    AF = mybir.ActivationFunctionType

    with tile.TileContext(nc) as tc:
        with tc.tile_pool(name="const", bufs=1) as const_pool:
            statesQ = const_pool.tile([64, 2048], BF16, name="statesQ_t")[:]
            Wl1 = const_pool.tile([48, 128], BF16, name="Wl1_t")[:]
            W23 = const_pool.tile([128, 96], BF16, name="W23_t")[:]
            biases = const_pool.tile([128, 3], FP32, name="biases_t")[:]
            W2q = W23[:, 0:64]
            W3q = W23[:, 64:96]
            bias1 = biases[:, 0:1]
            bias2 = biases[:, 1:2]
            bias3 = biases[:, 2:3]
            A2dup = const_pool.tile([128, 2 * COLS], BF16, name="A2dup_t")[:]
            B2s = const_pool.tile([128, COLS], BF16, name="B2s_t")[:]

            nc.sync.dma_start(out=statesQ, in_=d_statesQ)
            nc.sync.dma_start(out=Wl1, in_=d_Wl1)
            nc.sync.dma_start(out=W23, in_=d_W23)
            nc.sync.dma_start(out=biases, in_=d_biases)

            # ---- Layer 1: A2/B2 = per-agent halves of the first linear layer.
            # A2[p, 32*sb + i]: p<64 -> even l-block (2sb), p>=64 -> odd (2sb+1)
            # via the shifted rows 32:48 of statesQ.
            with tc.tile_pool(name="abps", bufs=1, space="PSUM") as ab_pool:
                A2ps = ab_pool.tile([128, COLS], FP32, tag="a2", name="A2ps_t")[:]
                B2ps = ab_pool.tile([128, COLS], FP32, tag="b2", name="B2ps_t")[:]
                rhs_even = statesQ[0:16].rearrange("p (s c) -> p s c", s=32)
                rhs_odd = statesQ[32:48].rearrange("p (s c) -> p s c", s=32)
                for w_lo, ps in ((0, A2ps), (64, B2ps)):
                    for half, rhs in ((0, rhs_even), (1, rhs_odd)):
                        lhsT = Wl1[32 * half : 32 * half + 16, w_lo : w_lo + 64]
                        for sbh in (0, 1):
                            nc.tensor.matmul(
                                ps[64 * half : 64 * half + 64, 512 * sbh : 512 * sbh + 512],
                                lhsT,
                                rhs[:, 16 * sbh : 16 * sbh + 16, 0:32],
                            )
                # Evict A2 twice (duplicated pairs so the later broadcast add
                # keeps an innermost unit stride), folding in b1; B2 plain.
                dupview = A2dup.rearrange("p (c two) -> p two c", two=2)
                nc.scalar.activation(dupview[:, 0], A2ps, AF.Identity, bias=bias1)
                nc.scalar.activation(dupview[:, 1], A2ps, AF.Identity, bias=bias1)
                nc.vector.tensor_copy(B2s, B2ps)

            with (
                tc.tile_pool(name="work", bufs=4) as work_pool,
                tc.tile_pool(name="sigp", bufs=2) as sig_pool,
                tc.tile_pool(name="l2ps", bufs=2, space="PSUM") as l2_pool,
                tc.tile_pool(name="l3ps", bufs=2, space="PSUM") as l3_pool,
            ):
                # Software pipeline, 1 sb deep: PE order is
                # L2(0), L2(1), L3(0), L2(2), L3(1), ... so L3(sb) (which
                # needs S2(sb)'s eviction) never blocks L2(sb+1) in the
                # strict-FIFO PE queue.
                h2_tiles = {}     # sb -> h2 AP
                psum3_tiles = {}  # pair -> psum3 AP
                sig_tiles = {}    # pairgroup -> sig AP

                def emit_s1(pair):
                    h1pre = work_pool.tile([128, 2 * COLS], BF16, tag="h1pre", name="h1pre_t")[:]
                    h1 = work_pool.tile([128, 2 * COLS], BF16, tag="h1", name="h1_t")[:]
                    a_in = (
                        A2dup[:, 128 * pair : 128 * pair + 128]
                        .rearrange("p (s i two) -> p s i two", s=2, two=2)
                        .unsqueeze(3)
                        .broadcast_to([128, 2, 32, 16, 2])
                    )
                    b_in = (
                        B2s[:, 64 * pair : 64 * pair + 64]
                        .rearrange("p (s jh jl) -> p s jh jl", s=2, jl=2)
                        .unsqueeze(2)
                        .broadcast_to([128, 2, 32, 16, 2])
                    )
                    h1pre_v = h1pre.rearrange(
                        "p (s i jh jl) -> p s i jh jl", s=2, i=32, jl=2
                    )
                    nc.vector.tensor_add(h1pre_v, a_in, b_in)
                    nc.vector.tensor_scalar_max(h1, h1pre, 0.0)
                    return h1

                def emit_l2_s2(sb, hk):
                    # Column-half a: evicted by ScalarE; column-half b by
                    # VectorE. Separate PSUM banks + separate h2 tiles keep
                    # each L3 chunk dependent on exactly one evicting engine.
                    psum2a = l2_pool.tile([128, 512], FP32, tag="l2a", name="psum2a_t")[:]
                    psum2b = l2_pool.tile([128, 512], FP32, tag="l2b", name="psum2b_t")[:]
                    nc.tensor.matmul(psum2a[0:64], W2q[0:64], hk[0:64, 0:512], tile_position=(0, 0))
                    nc.tensor.matmul(psum2a[64:128], W2q[64:128], hk[64:128, 0:512], tile_position=(64, 64))
                    nc.tensor.matmul(psum2b[64:128], W2q[0:64], hk[0:64, 512:1024], tile_position=(0, 64))
                    nc.tensor.matmul(psum2b[0:64], W2q[64:128], hk[64:128, 512:1024], tile_position=(64, 0))
                    h2a = work_pool.tile([128, 512], BF16, tag="h2a", name="h2a_t")[:]
                    h2b = work_pool.tile([128, 512], BF16, tag="h2b", name="h2b_t")[:]
                    nc.scalar.activation(h2a, psum2a, AF.Relu, bias=bias2)
                    nc.vector.tensor_scalar(h2b, psum2b, bias2, 0.0, add, max_)
                    h2_tiles[sb] = (h2a, h2b)

                def emit_l3(sb):
                    pair, k = divmod(sb, 2)
                    if k == 0:
                        psum3_tiles[pair] = l3_pool.tile([128, COLS], FP32, tag="l3", name="psum3_t")[:]
                    psum3 = psum3_tiles[pair]
                    h2a, h2b = h2_tiles.pop(sb)
                    ck = slice(512 * k, 512 * k + 512)
                    nc.tensor.matmul(psum3[0:32, ck], W3q[0:64], h2a[0:64], tile_position=(0, 0))
                    nc.tensor.matmul(psum3[32:64, ck], W3q[64:128], h2a[64:128], tile_position=(64, 32))
                    nc.tensor.matmul(psum3[64:96, ck], W3q[64:128], h2b[64:128], tile_position=(64, 64))
                    nc.tensor.matmul(psum3[96:128, ck], W3q[0:64], h2b[0:64], tile_position=(0, 96))

                def emit_sigmoid_dma(pair):
                    if pair % 2 == 0:
                        sig_tiles[pair // 2] = sig_pool.tile([128, 2 * COLS], FP32, tag="sig", name="sig_t")[:]
                    sig2 = sig_tiles[pair // 2]
                    psum3 = psum3_tiles.pop(pair)
                    half = COLS * (pair % 2)
                    nc.scalar.activation(
                        sig2[:, half : half + COLS], psum3, AF.Sigmoid, bias=bias3
                    )
                    if pair % 2 == 1:
                        for g in range(4):
                            nc.sync.dma_start(
                                out=d_out[pair // 2, g],
                                in_=sig2[32 * g : 32 * g + F],
                            )

                h1_cur = None
                for sb in range(NSB):
                    pair, k = divmod(sb, 2)
                    if k == 0:
                        h1_cur = emit_s1(pair)
                    if sb >= 1:
                        emit_l3(sb - 1)
                        if sb % 2 == 0:  # sb-1 was odd: its pair is complete
                            emit_sigmoid_dma((sb - 1) // 2)
                    emit_l2_s2(sb, h1_cur[:, COLS * k : COLS * k + COLS])
                emit_l3(NSB - 1)
                emit_sigmoid_dma(NPAIR - 1)

    nc.compile()
    input_names = ["statesQ", "Wl1", "W2q", "W3q", "bias1", "bias2", "bias3"]
    return nc, input_names


def get_program():
    global _PROGRAM
    if _PROGRAM is None:
        _PROGRAM = _build_program()
    return _PROGRAM


def make_inputs(states, W1, b1, W2, b2, W3, b3):
    """Host-side prep: per-core statesQ + shared packed weights/biases."""
    states = np.asarray(states, np.float32)
    W1 = np.asarray(W1, np.float32)
    W2 = np.asarray(W2, np.float32)
    W3 = np.asarray(W3, np.float32)
    b1 = np.asarray(b1, np.float32)
    b2 = np.asarray(b2, np.float32)
    b3 = np.asarray(b3, np.float32)

    Wl1 = np.zeros((48, 128), NP_BF16)
    Wl1[0:16, 0:64] = W1[:D].astype(NP_BF16)
    Wl1[0:16, 64:128] = W1[D:].astype(NP_BF16)
    Wl1[32:48, 0:64] = W1[:D].astype(NP_BF16)
    Wl1[32:48, 64:128] = W1[D:].astype(NP_BF16)

    W23 = np.zeros((128, 96), NP_BF16)
    W23[0:64, 0:64] = W2.astype(NP_BF16)
    W23[64:128, 0:64] = W2.astype(NP_BF16)
    W23[0:64, 64:72] = W3.astype(NP_BF16)
    W23[64:128, 64:72] = W3.astype(NP_BF16)

    biases = np.zeros((128, 3), np.float32)
    biases[:, 0] = np.tile(b1, 2)
    biases[:, 1] = np.tile(b2, 2)
    biases[:, 2] = np.tile(np.concatenate([b3, np.zeros(24, np.float32)]), 4)

    shared = {"Wl1": Wl1, "W23": W23, "biases": biases}

    in_maps = []
    for c in range(NCORES):
        # statesT[d, 32*l + i] = states[c, l, i, d]
        statesT = states[c].reshape(L * N, D).T.astype(NP_BF16)
        sQ = np.zeros((64, 2048), NP_BF16)
        sQ[0:16] = statesT
        sQ[32:48, : 2048 - 32] = statesT[:, 32:]
        in_maps.append({"statesQ": sQ, **shared})
    return in_maps


def decode_output(raw):
    """Invert the device output layout -> [L, N, N, F] for one core.

    raw: [NPAIR//2, 4, F, 2, COLS]; group g = 2*colhalf + block_parity,
    kp = pair%2, col = 512*k + q holds inner-sb k;
    l = 8*pg + 4*kp + 2*k + parity, pair-col = 512*colhalf + q = 32*i + j.
    """
    ov = raw.reshape(8, 2, 2, F, 2, 2, 512)          # [pg, h, par, f, kp, k, q]
    ov = ov.transpose(0, 4, 5, 2, 1, 6, 3)           # [pg, kp, k, par, h, q, f]
    return np.ascontiguousarray(ov.reshape(L, N, N, F))


def _ensure_ntff_hook():
    """Best-effort shim for the missing antenv.axon_hooks module so
    run_bass_kernel_spmd(trace=True) can capture NTFF profiles under axon."""
    import types

    try:
        from antenv.axon_hooks import get_axon_ntff_profile_hook  # noqa: F401
        return
    except ImportError:
        pass
    try:
        if "/root/.axon_site" not in sys.path:
            sys.path.insert(0, "/root/.axon_site")
        from trn_agent_boot.trn_boot import _ntff_profile_via_ctypes

        hook = _ntff_profile_via_ctypes("/opt/axon/libaxon_pjrt.so")
        import antenv

        mod = types.ModuleType("antenv.axon_hooks")
        mod._hook = hook
        mod.set_axon_ntff_profile_hook = lambda h: setattr(mod, "_hook", h)
        mod.get_axon_ntff_profile_hook = lambda: mod._hook
        sys.modules["antenv.axon_hooks"] = mod
        antenv.axon_hooks = mod
    except Exception as e:  # tracing is optional; never break the run
        print(f"ntff hook shim failed: {e}", file=sys.stderr)


def kernel(states, W1, b1, W2, b2, W3, b3):
    global LAST_RESULT
    nc, _ = get_program()
    if os.environ.get("KERNEL_TRACE"):
        _ensure_ntff_hook()
    in_maps = make_inputs(states, W1, b1, W2, b2, W3, b3)
    res = run_bass_kernel_spmd(
        nc,
        in_maps,
        core_ids=list(range(NCORES)),
        trace=bool(os.environ.get("KERNEL_TRACE")),
    )
    LAST_RESULT = res
    out = np.empty((B, L, N, N, F), np.float32)
    for c in range(NCORES):
        out[c] = decode_output(res.results[c]["out"])
    return out

